# revision 34
# baseline (speedup 1.0000x reference)
"""GQA attention (B=2, S=2048, H=2048, 32 heads / 8 KV groups, rope, causal-masked
softmax, output projection) distributed over 8 Trainium2 NeuronCores.

Sharding: data parallel over batch (2) x tensor parallel over KV groups (4 group-pairs).
Core c handles batch c//4 and KV groups {2*(c%4), 2*(c%4)+1} (= 8 q heads). Each core
computes its partial output projection (attn_out_shard @ wo_cols_shard.T); the host
sums the 4 partials per batch (the "all-reduce") and adds bo.

v2 layout/schedule:
- Projection chunks (512 s-cols) interleave with attention stages (attn(qt) only
  needs proj chunks <= qt in causal mode) in ONE psum pool of [128,1024] slots so
  the tensor engine never drains (HAM stays at full clock).
- Head dims permuted per 32-block as [e0..15 | o0..15] so the rope cross-term is a
  single stream_shuffle (swap 16-halves); rope runs on 4 full-width DVE ops in bf16.
- Scores/AV matmuls causal-trimmed to skip fully-masked 128-col blocks; the diagonal
  128x128 triangle is zeroed by one small bf16 multiply per diag tile.
- Q/K/V/eT/avq/wo in bf16 (same PE stream rate, 2x DVE, half SBUF); x/wq/wkv f32r.
"""
import sys

for _p in ("/opt/trn_rl_repo",):
    if _p not in sys.path:
        sys.path.append(_p)

import numpy as np

S = 2048
H = 2048
HD = 64
NQT = 4          # s_q tiles of 512
NKT = 16         # s_k tiles of 128

_CACHE = {}


def _build(mode, has_bq, has_bk, has_bv):
    import concourse.bass as bass  # noqa: F401
    import concourse.mybir as mybir
    import concourse.tile as tile
    from concourse import bacc
    from concourse.masks import make_identity

    f32 = mybir.dt.float32
    f32r = mybir.dt.float32r
    bf16 = mybir.dt.bfloat16
    AF = mybir.ActivationFunctionType
    ALU = mybir.AluOpType

    nc = bacc.Bacc("TRN2", target_bir_lowering=False, debug=False)
    xT = nc.dram_tensor("xT", [H, S], bf16, kind="ExternalInput")
    wqT = nc.dram_tensor("wqT", [H, 512], bf16, kind="ExternalInput")
    wkvT = nc.dram_tensor("wkvT", [H, 256], bf16, kind="ExternalInput")
    woR = nc.dram_tensor("woR", [512, H], bf16, kind="ExternalInput")
    COSd = nc.dram_tensor("COSx", [128, S], bf16, kind="ExternalInput")
    SINd = nc.dram_tensor("SINx", [128, S], bf16, kind="ExternalInput")
    outd = nc.dram_tensor("out", [S, H], f32, kind="ExternalOutput")
    maskd = nc.dram_tensor("maskT", [S, S], f32, kind="ExternalInput") if mode == "generic" else None
    bqd = nc.dram_tensor("bq", [512, 1], f32, kind="ExternalInput") if has_bq else None
    bkvd = nc.dram_tensor("bkv", [256, 1], f32, kind="ExternalInput") if (has_bk or has_bv) else None

    causal = mode == "causal"
    # swap 16-halves within each 32-partition quadrant (rope pair exchange)
    swap16 = [(i + 16) % 32 for i in range(32)]

    with tile.TileContext(nc) as tc:
        with (
            tc.tile_pool(name="const", bufs=1) as cstp,
            tc.tile_pool(name="wts", bufs=1) as wts,
            tc.tile_pool(name="xs", bufs=32) as xsp,
            tc.tile_pool(name="per", bufs=1) as per,
            tc.tile_pool(name="rtmp", bufs=3) as rtp,
            tc.tile_pool(name="et", bufs=6) as etp,
            tc.tile_pool(name="outs", bufs=3) as outp,
            tc.tile_pool(name="mks", bufs=2) as mkp,
            tc.tile_pool(name="ps", bufs=4, space="PSUM") as psp,
        ):
            # ---- inputs, k-interleaved so the first proj matmuls start early ----
            x_t = {}
            wq_t = [wts.tile([128, 512], bf16, tag=f"wq{k}", name=f"wq_t{k}") for k in range(16)]
            wkv_t = [wts.tile([128, 256], bf16, tag=f"wkv{k}", name=f"wkv_t{k}") for k in range(16)]
            for k in range(16):
                x_t[(0, k)] = xsp.tile([128, 512], bf16, tag="x", name=f"x0_{k}")
                nc.sync.dma_start(x_t[(0, k)][:], xT[128 * k:128 * (k + 1), 0:512])
                nc.sync.dma_start(wq_t[k][:], wqT[128 * k:128 * (k + 1), :])
                nc.sync.dma_start(wkv_t[k][:], wkvT[128 * k:128 * (k + 1), :])

            COS = cstp.tile([128, S], bf16, tag="cos")
            SIN = cstp.tile([128, S], bf16, tag="sin")
            nc.sync.dma_start(COS[:], COSd[:])
            nc.sync.dma_start(SIN[:], SINd[:])
            identb = cstp.tile([128, 128], bf16, tag="ident")
            make_identity(nc, identb[:])
            if causal:
                # TriA.T @ identb == -1e9 * strict-upper(col<row): one extra
                # accumulate-matmul adds the causal mask to a diagonal score
                # block inside PSUM - no DVE/GpSimd hop in the exp->AV chain.
                TriA = cstp.tile([128, 128], bf16, tag="tria")
                nc.gpsimd.memset(TriA[:], -1e9)
                nc.gpsimd.affine_select(
                    out=TriA[:], in_=TriA[:], pattern=[[1, 128]],
                    compare_op=ALU.is_gt, fill=0.0, base=0, channel_multiplier=-1)
            if has_bq:
                bq_t = [cstp.tile([128, 1], f32, tag=f"bq{m}", name=f"bq_t{m}") for m in range(4)]
                for m in range(4):
                    nc.sync.dma_start(bq_t[m][:], bqd[128 * m:128 * (m + 1), :])
            if has_bk or has_bv:
                bk_t = cstp.tile([128, 1], f32, tag="bkt")
                bv_t = cstp.tile([128, 1], f32, tag="bvt")
                nc.sync.dma_start(bk_t[:], bkvd[0:128, :])
                nc.sync.dma_start(bv_t[:], bkvd[128:256, :])

            # persistent intermediates
            QTrot = [per.tile([128, S], bf16, tag=f"qtrot{m}", name=f"QTrot{m}") for m in range(4)]
            KTrot = per.tile([128, S], bf16, tag="ktrot")
            # V with a ones column per kt-block: [g0 v64 | 1 | g1 v64 | 1] x 16 kt
            Vp = per.tile([128, 130 * NKT], bf16, tag="vp")
            nc.gpsimd.memset(Vp[:], 1.0)  # ones columns at 130*kt+{64,129} survive
            den = [[per.tile([128, 512], f32, tag=f"den{p}_{h}", name=f"den{p}_{h}")
                    for h in range(2)] for p in range(2)]
            den_r = [[per.tile([128, 512], f32, tag=f"denr{p}_{h}", name=f"denr{p}_{h}")
                      for h in range(2)] for p in range(2)]
            ones1 = per.tile([1, 512], f32, tag="ones1")
            nc.gpsimd.memset(ones1[:], 1.0)
            den_rb = [[per.tile([128, 512], bf16, tag=f"denrb{p}_{h}", name=f"denrb{p}_{h}")
                       for h in range(2)] for p in range(2)]
            sel = [per.tile([128, 64], bf16, tag=f"sel{m}", name=f"sel{m}") for m in range(4)]
            for m in range(4):
                nc.gpsimd.memset(sel[m][:], 0.0)
                nc.gpsimd.memset(sel[m][32 * m:32 * m + 1, :], 1.0)
            for p in range(2):
                for h in range(2):
                    nc.gpsimd.memset(den[p][h][:], 1.0)
                    nc.gpsimd.memset(den_r[p][h][:], 1.0)
            VTt = per.tile([128, 512], bf16, tag="vtt")
            avq_all = [[per.tile([128, 512], bf16, tag=f"avq{q}_{m}", name=f"avq{q}_{m}")
                        for m in range(4)] for q in range(4)]  # one set per qt
            # output-projection weights (bf16), deferred in program order
            wo_t = [wts.tile([128, S], bf16, tag=f"wor{k}", name=f"wo_t{k}") for k in range(4)]
            for k in range(4):
                nc.sync.dma_start(wo_t[k][:], woR[128 * k:128 * (k + 1), :])

            def rope_evict(ps_region, ssl, dst):
                """psum [128,512] f32 -> dst[:, ssl] bf16 rotated.

                SIN arrives host-pre-shuffled, so both multiplies read the psum
                region directly (slot frees after 2 DVE reads); the bf16
                product is then partition-shuffled in SBUF and GpSimd does the
                final add (idle otherwise — rope results aren't needed until
                the next stage)."""
                t1 = rtp.tile([128, 512], bf16, tag="t1", name="t1")
                nc.vector.tensor_tensor(t1[:], ps_region, COS[:, ssl], ALU.mult)
                t2p = rtp.tile([128, 512], bf16, tag="t2p", name="t2p")
                nc.vector.tensor_tensor(t2p[:], ps_region, SIN[:, ssl], ALU.mult)
                t2 = rtp.tile([128, 512], bf16, tag="t2", name="t2")
                nc.vector.stream_shuffle(t2[:], t2p[:], swap16)
                nc.gpsimd.tensor_tensor(dst[:, ssl], t1[:], t2[:], ALU.add)

            def prefetch_x(sc):
                for k in range(16):
                    x_t[(sc, k)] = xsp.tile([128, 512], bf16, tag="x",
                                            name=f"x{sc}_{k}")
                    nc.sync.dma_start(x_t[(sc, k)][:],
                                      xT[128 * k:128 * (k + 1),
                                         512 * sc:512 * (sc + 1)])

            def proj_steps(sc):
                """Generator: 3 units x (16 k-steps + rope tail), yielding
                after every ~2 matmuls so the driver can interleave streams."""
                ssl = slice(512 * sc, 512 * (sc + 1))
                for u in (2, 0, 1):
                    pp = psp.tile([128, 1024], f32, tag="dual", bufs=4,
                                  name=f"pp{sc}_{u}")
                    for k in range(16):
                        xk = x_t[(sc, k)]
                        st, sp = (k == 0), (k == 15)
                        if u < 2:
                            for half in range(2):
                                m = 2 * u + half
                                nc.tensor.matmul(pp[:, 512 * half:512 * (half + 1)],
                                                 wq_t[k][:, 128 * m:128 * (m + 1)],
                                                 xk[:], start=st, stop=sp)
                        else:
                            nc.tensor.matmul(pp[:, 0:512], wkv_t[k][:, 0:128], xk[:],
                                             start=st, stop=sp)
                            nc.tensor.matmul(pp[:, 512:1024], wkv_t[k][:, 128:256],
                                             xk[:], start=st, stop=sp)
                        yield
                    if u < 2:
                        for half in range(2):
                            m = 2 * u + half
                            reg = pp[:, 512 * half:512 * (half + 1)]
                            if has_bq:
                                nc.vector.tensor_scalar_add(reg, reg, bq_t[m][:])
                            rope_evict(reg, ssl, QTrot[m])
                            yield
                    else:
                        regK = pp[:, 0:512]
                        if has_bk:
                            nc.vector.tensor_scalar_add(regK, regK, bk_t[:])
                        rope_evict(regK, ssl, KTrot)
                        yield
                        regV = pp[:, 512:1024]
                        if has_bv:
                            nc.vector.tensor_scalar_add(regV, regV, bv_t[:])
                        nc.vector.tensor_copy(VTt[:], regV)
                        vps = psp.tile([128, 512], bf16, tag="dual", name=f"vps{sc}")
                        for j in range(4):
                            nc.tensor.transpose(vps[:, 128 * j:128 * (j + 1)],
                                                VTt[:, 128 * j:128 * (j + 1)], identb[:])
                        yield
                        for j in range(4):
                            kt = 4 * sc + j
                            dst = Vp[:, 130 * kt:130 * kt + 130].rearrange(
                                "p (two x) -> p two x", two=2)[:, :, 0:64]
                            src = vps[:, 128 * j:128 * (j + 1)].rearrange(
                                "p (two x) -> p two x", two=2)
                            nc.vector.tensor_copy(dst, src)
                        yield

            def attn_m_steps(qt, m):
                """Generator: one m-block of the scores/exp/AV pipeline,
                yielding per (pr, hloc) iteration."""
                qsl = slice(512 * qt, 512 * (qt + 1))
                n_kt = 4 * qt + 4 if causal else NKT
                avq = avq_all[qt]
                if True:
                    # hloc 0/1 = heads m (group 0, rows 0:64) / m+4 (group 1)
                    av = psp.tile([128, 1024], f32, tag="dual", name=f"av{qt}_{m}")
                    for pr in range(n_kt // 2):
                        for hloc in range(2):
                            g = hloc
                            qb = 64 * hloc
                            sc2 = psp.tile([128, 1024], f32, tag="dual", bufs=4,
                                           name="sc2")
                            eT = etp.tile([128, 1024], bf16, tag="eT", name="eT")
                            los = []
                            for half in range(2):
                                kt = 2 * pr + half
                                t = kt - 4 * qt
                                diag = causal and 0 <= t <= 3
                                lo = 128 * t if diag else 0
                                los.append(lo)
                                # half1 scores run full-width so one ACT covers
                                # the pair (masked-junk cols skipped by AV)
                                slo = lo if half == 0 else 0
                                nc.tensor.matmul(
                                    sc2[:, 512 * half + slo:512 * (half + 1)],
                                    KTrot[64 * g:64 * g + 64, 128 * kt:128 * (kt + 1)],
                                    QTrot[m][qb:qb + 64, 512 * qt + slo:512 * (qt + 1)],
                                    start=True, stop=not diag)
                                if diag:
                                    # add -1e9 strict-upper triangle to the
                                    # partial 128-col block inside PSUM
                                    nc.tensor.matmul(
                                        sc2[:, 512 * half + lo:512 * half + lo + 128],
                                        TriA[:], identb[:], start=False, stop=True)
                            if mode == "generic":
                                for half in range(2):
                                    kt = 2 * pr + half
                                    mk = mkp.tile([128, 512], f32, tag="mk", name="mk")
                                    nc.sync.dma_start(mk[:], maskd[128 * kt:128 * (kt + 1), qsl])
                                    stt = mkp.tile([128, 512], f32, tag="stt", name="stt")
                                    nc.vector.scalar_tensor_tensor(
                                        stt[:], sc2[:, 512 * half:512 * (half + 1)], 0.125,
                                        mk[:], ALU.mult, ALU.add)
                                    nc.scalar.activation(
                                        eT[:, 512 * half:512 * (half + 1)], stt[:],
                                        AF.Exp, scale=1.0)
                            else:
                                lo0 = los[0]
                                nc.scalar.activation(eT[:, lo0:1024], sc2[:, lo0:1024],
                                                     AF.Exp, scale=0.125)
                            for half in range(2):
                                kt = 2 * pr + half
                                lo = los[half]
                                nc.tensor.matmul(
                                    av[0:65, 512 * hloc + lo:512 * (hloc + 1)],
                                    Vp[:, 130 * kt + 65 * g:130 * kt + 65 * g + 65],
                                    eT[:, 512 * half + lo:512 * (half + 1)],
                                    start=(kt == 0), stop=(kt == n_kt - 1))
                            yield
                    for hloc in range(2):
                        qb = 64 * hloc
                        # evict unnormalized AV (bf16) and its denominator row
                        nc.vector.tensor_scalar_mul(avq[m][qb:qb + 64, :],
                                                    av[0:64, 512 * hloc:512 * (hloc + 1)], 1.0)
                        nc.vector.tensor_tensor(den[qt % 2][hloc][32 * m:32 * m + 1, :],
                                                av[64:65, 512 * hloc:512 * (hloc + 1)],
                                                ones1[:], ALU.mult)
                    yield

            def attn_norm_steps(qt):
                avq = avq_all[qt]
                par = qt % 2
                for hloc in range(2):
                    nc.vector.reciprocal_approx_fast(den_r[par][hloc][:], den[par][hloc][:])
                    nc.vector.tensor_copy(den_rb[par][hloc][:], den_r[par][hloc][:])
                yield
                for m in range(4):
                    rcb = psp.tile([128, 1024], f32, tag="dual", name="rcb")
                    for hloc in range(2):
                        nc.tensor.matmul(rcb[0:64, 512 * hloc:512 * (hloc + 1)],
                                         sel[m][:], den_rb[par][hloc][:],
                                         start=True, stop=True)
                        qb = 64 * hloc
                        nc.vector.tensor_tensor(avq[m][qb:qb + 64, :], avq[m][qb:qb + 64, :],
                                                rcb[0:64, 512 * hloc:512 * (hloc + 1)],
                                                ALU.mult)
                    yield

            def oproj_steps(qt):
                """Generator: output projection, one (j, n-pair) block per step."""
                avq = avq_all[qt]
                for j in range(4):
                    mm = 4 * qt + j
                    for np_ in range(2):
                        op = psp.tile([128, 1024], f32, tag="dual", name="op")
                        for half in range(2):
                            n = 2 * np_ + half
                            nsl = slice(512 * n, 512 * (n + 1))
                            for k in range(4):
                                nc.tensor.matmul(op[:, 512 * half:512 * (half + 1)],
                                                 avq[k][:, 128 * j:128 * (j + 1)],
                                                 wo_t[k][:, nsl], start=(k == 0), stop=(k == 3))
                        ot = outp.tile([128, 1024], f32, tag="ot", name="ot")
                        nc.vector.tensor_copy(ot[:], op[:])
                        nc.sync.dma_start(outd[128 * mm:128 * (mm + 1),
                                               1024 * np_:1024 * (np_ + 1)], ot[:])
                        yield

            def drain(*gens):
                its = [iter(g) for g in gens]
                while its:
                    alive = []
                    for it in its:
                        try:
                            next(it)
                            alive.append(it)
                        except StopIteration:
                            pass
                    its = alive

            def chain(*gens):
                for g in gens:
                    yield from g

            def attn_all(qt):
                return chain(*[attn_m_steps(qt, m) for m in range(4)],
                             attn_norm_steps(qt))

            if causal:
                # Software pipeline, round-robin interleaved at ~2-matmul
                # granularity: stage s mixes proj(sc=s) k-steps, attention
                # iterations of qt=s-1, output-projection blocks of qt=s-2,
                # and qt3's first m-block rides in stage 3 to balance the
                # ACT-heavy tail.
                prefetch_x(1)
                drain(proj_steps(0))
                prefetch_x(2)
                drain(proj_steps(1), attn_all(0))
                prefetch_x(3)
                drain(proj_steps(2), attn_all(1), oproj_steps(0))
                drain(proj_steps(3), attn_all(2), oproj_steps(1))
                drain(attn_all(3), oproj_steps(2))
                drain(oproj_steps(3))
            else:
                for sc in range(4):
                    if sc + 1 < 4:
                        prefetch_x(sc + 1)
                    drain(proj_steps(sc))
                for qt in range(NQT):
                    drain(attn_all(qt))
                    drain(oproj_steps(qt))

    nc.compile()
    return nc


# within each 32-dim block of a head: evens first 16, odds last 16 ->
# rope pair (e_i, o_i) sits 16 partitions apart; the cross-term is a
# stream_shuffle swapping 16-halves of every 32-partition quadrant.
_PERM64 = np.concatenate([
    np.arange(0, 32, 2), np.arange(1, 32, 2),
    np.arange(32, 64, 2), np.arange(33, 64, 2)])
# Q-tile m holds local heads (m, m+4) so each head's partition base (0/64) matches
# its KV group's base in KTrot (group g at rows 64g) - matmul requires equal bases.
_HEADS_ORDER = np.array([0, 4, 1, 5, 2, 6, 3, 7])


def _freq_rows():
    """cos/sin row patterns for the 128-partition blocked layout.

    Returns (cos_idx, sin_idx, sin_sign): for partition p (mod 64 layout),
    COS[p] = cos[pair(p)], SIN[p] = sign * sin[pair(p)].
    """
    pair = np.empty(64, dtype=np.int64)
    sign = np.empty(64, dtype=np.float32)
    for d in range(64):
        blk, sub = divmod(d, 32)
        if sub < 16:          # even slot
            pair[d] = 16 * blk + sub
            sign[d] = -1.0
        else:                 # odd slot
            pair[d] = 16 * blk + (sub - 16)
            sign[d] = 1.0
    return pair, sign


def _prep_core(c, x, freqs_cis, mask, wq, bq, wk, bk, wv, bv, wo, mode,
               has_bq, has_bk, has_bv):
    import ml_dtypes
    b, gp = divmod(c, 4)
    f = np.float32
    bf = ml_dtypes.bfloat16
    xT = np.ascontiguousarray(x[b].T, dtype=bf)
    wq_c = wq[512 * gp:512 * (gp + 1)].reshape(8, 64, H)[_HEADS_ORDER][:, _PERM64, :].reshape(512, H)
    wqT = np.ascontiguousarray(wq_c.T, dtype=bf)
    wk_c = wk[128 * gp:128 * (gp + 1)].reshape(2, 64, H)[:, _PERM64, :].reshape(128, H)
    wv_c = wv[128 * gp:128 * (gp + 1)]
    wkvT = np.ascontiguousarray(np.concatenate([wk_c, wv_c], 0).T, dtype=bf)
    woR = wo[:, 512 * gp:512 * (gp + 1)].T.reshape(8, 64, H)[_HEADS_ORDER].reshape(512, H)
    woR = np.ascontiguousarray(woR, dtype=bf)
    cosv = freqs_cis[:, 0::2].astype(np.float64)   # (S, 32)
    sinv = freqs_cis[:, 1::2].astype(np.float64)
    pair, sign = _freq_rows()
    cos64 = cosv[:, pair].T                        # (64, S)
    sin64 = (sinv[:, pair] * sign[None, :]).T
    COS = np.tile(cos64, (2, 1)).astype(bf)        # (128, S)
    SIN = np.tile(sin64, (2, 1)).astype(bf)
    # pre-shuffle SIN rows so the kernel can multiply before the partition
    # shuffle: t2 = shuffle(ps * SIN_pre) == shuffle(ps) * SIN
    p = np.arange(128)
    SIN = SIN[(p // 32) * 32 + (p % 32 + 16) % 32]
    m = {"xT": xT, "wqT": wqT, "wkvT": wkvT, "woR": woR,
         "COSx": np.ascontiguousarray(COS), "SINx": np.ascontiguousarray(SIN)}
    if mode == "generic":
        m["maskT"] = np.ascontiguousarray(mask.T, dtype=f)
    if has_bq:
        bq_c = bq[512 * gp:512 * (gp + 1)].reshape(8, 64)[_HEADS_ORDER][:, _PERM64].reshape(512, 1)
        m["bq"] = np.ascontiguousarray(bq_c, dtype=f)
    if has_bk or has_bv:
        bk_c = bk[128 * gp:128 * (gp + 1)].reshape(2, 64)[:, _PERM64].reshape(128)
        bv_c = bv[128 * gp:128 * (gp + 1)]
        m["bkv"] = np.ascontiguousarray(np.concatenate([bk_c, bv_c]).reshape(256, 1), dtype=f)
    return m


def _detect_mode(mask):
    causal = np.where(np.tril(np.ones((S, S), dtype=bool)), np.float32(0.0), np.float32(-1e9))
    if np.array_equal(mask, causal):
        return "causal"
    if not np.any(mask):
        return "zeros"
    return "generic"


def _run(inputs, trace=False):
    from concourse import bass_utils
    x = np.asarray(inputs["x"], dtype=np.float32)
    freqs_cis = np.asarray(inputs["freqs_cis"], dtype=np.float32)
    mask = np.asarray(inputs["mask"], dtype=np.float32)
    wq = np.asarray(inputs["wq"], dtype=np.float32)
    bq = np.asarray(inputs["bq"], dtype=np.float32)
    wk = np.asarray(inputs["wk"], dtype=np.float32)
    bk = np.asarray(inputs["bk"], dtype=np.float32)
    wv = np.asarray(inputs["wv"], dtype=np.float32)
    bv = np.asarray(inputs["bv"], dtype=np.float32)
    wo = np.asarray(inputs["wo"], dtype=np.float32)
    bo = np.asarray(inputs["bo"], dtype=np.float32)

    mode = _detect_mode(mask)
    has_bq = bool(np.any(bq))
    has_bk = bool(np.any(bk))
    has_bv = bool(np.any(bv))
    key = (mode, has_bq, has_bk, has_bv)
    if key not in _CACHE:
        _CACHE[key] = _build(*key)
    nc = _CACHE[key]

    in_maps = [
        _prep_core(c, x, freqs_cis, mask, wq, bq, wk, bk, wv, bv, wo, mode,
                   has_bq, has_bk, has_bv)
        for c in range(8)
    ]
    res = bass_utils.run_bass_kernel_spmd(nc, in_maps, core_ids=list(range(8)), trace=trace)
    partials = np.stack([res.results[c]["out"] for c in range(8)], 0)  # (8, S, H)
    out = partials.reshape(2, 4, S, H).sum(axis=1) + bo[None, None, :]
    return out.astype(np.float32), res


def kernel(**inputs):
    out, _ = _run(inputs, trace=False)
    return out
